# revision 1
# baseline (speedup 1.0000x reference)
"""Trainium2 Bass kernel for a pre-LN transformer block (B=8,T=1024,C=1024,H=16,FF=4096).

Sharding: pure data-parallel over batch — B=8 equals the 8 NeuronCores, each core
runs the full block on one (T, C) slice; weights are replicated. No collectives.

Per-core layout strategy:
  - LayerNorm in token-major [tok(P), C], gains/biases folded into downstream
    weights on the host; normalized activations cast to bf16 and PE-transposed
    to feature-major h^T [C(P), tok] for use as matmul operands.
  - QKV: k-proj/q-proj produced feature-major (out^T = W_tile.T @ h^T),
    v produced token-major with a per-head ones column appended (for softmax
    denominators). The reference computes wei[i,j] = k_i . q_j, so the k-proj
    acts as the query side (1/sqrt(D) folded into it on the host).
  - Attention: S^T[j,i] tiles via K=64 matmuls, causal tile skipping,
    exp without max-subtraction (scores are O(3)), multiplicative triangular
    mask on diagonal tiles, PV token-major: out[i,65] = P^T_tile.T @ v_aug,
    column 64 = softmax denominator; normalize via per-partition reciprocal.
  - attn-proj / fc / mlp-proj as bf16 tiled matmuls with fp32 PSUM accum;
    residuals in fp32.
"""

import functools

import ml_dtypes
import numpy as np

import concourse.bass as bass
import concourse.mybir as mybir
import concourse.tile as tile
from concourse import bacc
from concourse.bass_utils import run_bass_kernel_spmd

bf16 = ml_dtypes.bfloat16
FP32 = mybir.dt.float32
BF16 = mybir.dt.bfloat16
AX = mybir.AxisListType
OP = mybir.AluOpType
AF = mybir.ActivationFunctionType

B, T, C, H = 8, 1024, 1024, 16
D = C // H          # 64
FF = 4 * C          # 4096
P = 128
NT = T // P         # 8 token tiles
NCT = C // P        # 8 channel tiles
NFT = FF // P       # 32 ff tiles
NIC = T // 512      # 2 i-chunks of 512


def _emit_ln(nc, sm, spool, x_tile, h_out):
    """h_out(bf16) = (x - mean(x)) * rsqrt(var(x) + 1e-5), rowwise over free dim."""
    width = x_tile.shape[-1]
    s = sm.tile([P, 1], FP32, tag="ln_s")
    ssq = sm.tile([P, 1], FP32, tag="ln_ssq")
    mu = sm.tile([P, 1], FP32, tag="ln_mu")
    var = sm.tile([P, 1], FP32, tag="ln_var")
    std = sm.tile([P, 1], FP32, tag="ln_std")
    rstd = sm.tile([P, 1], FP32, tag="ln_rstd")
    sq = spool.tile([P, C], FP32, tag="ln_sq")
    nc.vector.reduce_sum(s, x_tile, axis=AX.X)
    nc.scalar.activation(sq[:, :width], x_tile, AF.Square, accum_out=ssq)
    nc.vector.tensor_scalar_mul(mu, s, 1.0 / width)
    nc.vector.tensor_scalar_mul(var, ssq, 1.0 / width)
    # var = E[x^2] - mu^2  (fused: var = (var - mu*mu) via two ops)
    nc.vector.tensor_tensor(s, mu, mu, op=OP.mult)      # reuse s as mu^2
    nc.vector.tensor_tensor(var, var, s, op=OP.subtract)
    nc.scalar.activation(std, var, AF.Sqrt, bias=1e-5)
    nc.vector.reciprocal(rstd, std)
    nc.vector.tensor_scalar(h_out, x_tile, scalar1=mu, scalar2=rstd,
                            op0=OP.subtract, op1=OP.mult)


def emit_block(nc, tc):
    """Emit the whole per-core transformer block program.

    Emission order is a global software pipeline: the qk projections are
    interleaved with attention chunk 0 and the attention projection with
    attention chunk 1, so the ACT-bound softmax exp always has dense PE
    work (and a warm PE clock) running beside it.
    """
    x_d = nc.dram_tensor("x", [T, C], FP32, kind="ExternalInput").ap()
    wqk_d = nc.dram_tensor("wqk", [16, P, NCT, P], BF16, kind="ExternalInput").ap()
    wv_d = nc.dram_tensor("wv", [P, NCT, C], BF16, kind="ExternalInput").ap()
    wproj_d = nc.dram_tensor("wproj", [P, NCT, C], BF16, kind="ExternalInput").ap()
    wfc_d = nc.dram_tensor("wfc", [NFT, P, NCT, P], BF16, kind="ExternalInput").ap()
    wmp_d = nc.dram_tensor("wmp", [2, P, NFT, 512], BF16, kind="ExternalInput").ap()
    ident_d = nc.dram_tensor("ident", [P, P], BF16, kind="ExternalInput").ap()
    tri_d = nc.dram_tensor("tri01", [P, P], BF16, kind="ExternalInput").ap()
    out_d = nc.dram_tensor("out", [T, C], FP32, kind="ExternalOutput").ap()

    from contextlib import ExitStack
    with ExitStack() as top:
        cpool = top.enter_context(tc.tile_pool(name="const", bufs=1))
        ppool = top.enter_context(tc.tile_pool(name="persist", bufs=1))
        spool = top.enter_context(tc.tile_pool(name="stream", bufs=2))
        sm = top.enter_context(tc.tile_pool(name="small", bufs=4))
        aoT_pool = top.enter_context(tc.tile_pool(name="aoT", bufs=1))
        wp_pool = top.enter_context(tc.tile_pool(name="wproj", bufs=1))
        cs = top.enter_context(ExitStack())
        ps_mm = cs.enter_context(tc.tile_pool(name="ps_mm", bufs=2, space="PSUM"))
        ps_aux = ps_mm  # rb/tr fold into the mm pool (tag aux -> mm shape)

        ident = cpool.tile([P, P], BF16, tag="ident")
        tri01 = cpool.tile([P, P], BF16, tag="tri01")
        zero1 = cpool.tile([P, 1], FP32, tag="zero1")
        eps1 = cpool.tile([P, 1], FP32, tag="eps1")
        ones_row = cpool.tile([1, P], BF16, tag="ones_row")

        x2_sb = ppool.tile([P, NT, C], FP32, tag="x2")
        aoT = aoT_pool.tile([P, NCT, T], BF16, tag="aoT")
        wp = wp_pool.tile([P, NCT, C], BF16, tag="wproj")

        def emit_ln(x_tile, h_out):
            width = x_tile.shape[-1]
            s = sm.tile([P, 1], FP32, tag="ln_s")
            ssq = sm.tile([P, 1], FP32, tag="ln_ssq")
            mu = sm.tile([P, 1], FP32, tag="ln_mu")
            var = sm.tile([P, 1], FP32, tag="ln_var")
            std = sm.tile([P, 1], FP32, tag="ln_std")
            rstd = sm.tile([P, 1], FP32, tag="ln_rstd")
            sq = spool.tile([P, C], FP32, tag="ln_sq")
            nc.vector.reduce_sum(s, x_tile, axis=AX.X)
            nc.scalar.activation(sq[:, :width], x_tile, AF.Square, accum_out=ssq)
            nc.vector.tensor_scalar_mul(mu, s, 1.0 / width)
            nc.vector.tensor_scalar_mul(var, ssq, 1.0 / width)
            nc.vector.tensor_tensor(s, mu, mu, op=OP.mult)
            nc.vector.tensor_tensor(var, var, s, op=OP.subtract)
            nc.scalar.activation(std, var, AF.Sqrt, bias=1e-5)
            nc.vector.reciprocal(rstd, std)
            nc.vector.tensor_scalar(h_out, x_tile, scalar1=mu, scalar2=rstd,
                                    op0=OP.subtract, op1=OP.mult)

        with ExitStack() as attn_scope:
            ps_pv = attn_scope.enter_context(
                tc.tile_pool(name="ps_pv", bufs=2, space="PSUM"))
            ps_s = attn_scope.enter_context(
                tc.tile_pool(name="ps_s", bufs=4, space="PSUM"))
            qk_pool = attn_scope.enter_context(tc.tile_pool(name="qk", bufs=1))
            v_pool = attn_scope.enter_context(tc.tile_pool(name="v", bufs=1))
            hT_pool = attn_scope.enter_context(tc.tile_pool(name="hTp", bufs=1))
            wq_pool = attn_scope.enter_context(tc.tile_pool(name="wqkv", bufs=2))
            wv_pool = attn_scope.enter_context(tc.tile_pool(name="wvp", bufs=1))
            pt_pool = attn_scope.enter_context(tc.tile_pool(name="pt", bufs=24))
            rb_pool = attn_scope.enter_context(tc.tile_pool(name="rbp", bufs=2))

            qpT = qk_pool.tile([P, NCT, T], BF16, tag="qpT")  # q-proj^T (key role)
            kpT = qk_pool.tile([P, NCT, T], BF16, tag="kpT")  # k-proj^T (query role)
            v_aug = v_pool.tile([P, NT, H, D + 1], BF16, tag="vaug")
            hT = hT_pool.tile([P, NCT, T], BF16, tag="hT")

            # x tiles first (LN1 critical path), then constants + weights.
            xts = []
            for tt in range(NT):
                xt = spool.tile([P, C], FP32, tag="xin", name=f"xin{tt}")
                nc.sync.dma_start(xt[:], x_d[tt * P:(tt + 1) * P, :])
                xts.append(xt)
            nc.sync.dma_start(ident[:], ident_d)
            nc.sync.dma_start(tri01[:], tri_d)
            nc.gpsimd.memset(zero1[:], 0.0)
            nc.gpsimd.memset(eps1[:], 1e-5)
            nc.gpsimd.memset(ones_row[:], 1.0)
            nc.const_aps.aps[(FP32, 0.0)] = zero1[:]
            nc.const_aps.aps[(FP32, 1e-5)] = eps1[:]
            nc.gpsimd.memset(v_aug[:, :, :, D:D + 1], 1.0)
            wv_sb = wv_pool.tile([P, NCT, C], BF16, tag="wv")
            nc.sync.dma_start(wv_sb[:], wv_d)
            nc.sync.dma_start(wp[:], wproj_d)

            # ---- phase A: LN1 + transpose + v projection per token tile ----
            for tt in range(NT):
                ht = spool.tile([P, C], BF16, tag="h")
                emit_ln(xts[tt][:], ht[:])
                for ct in range(NCT):
                    ptr = ps_s.tile([P, P], BF16, tag="smm")
                    nc.tensor.transpose(ptr[:], ht[:, ct * P:(ct + 1) * P], ident[:])
                    nc.vector.tensor_copy(out=hT[:, ct, tt * P:(tt + 1) * P], in_=ptr[:])
                for fc2 in range(2):
                    pm = ps_mm.tile([P, 512], FP32, tag="mm")
                    for ct in range(NCT):
                        nc.tensor.matmul(pm[:], hT[:, ct, tt * P:(tt + 1) * P],
                                         wv_sb[:, ct, fc2 * 512:(fc2 + 1) * 512],
                                         start=(ct == 0), stop=(ct == NCT - 1))
                    nc.vector.tensor_copy(
                        out=v_aug[:, tt, fc2 * 8:(fc2 + 1) * 8, 0:D],
                        in_=pm[:].rearrange("p (h d) -> p h d", d=D))

            def emit_qk_ft(ft):
                wt = wq_pool.tile([P, NCT, P], BF16, tag="wqk", name=f"wqk{ft}")
                nc.sync.dma_start(wt[:], wqk_d[ft])
                dst = kpT if ft < 8 else qpT
                for tc2 in range(2):
                    pm = ps_mm.tile([P, 512], FP32, tag="mm", name=f"qk{ft}_{tc2}")
                    for ct in range(NCT):
                        nc.tensor.matmul(pm[:], wt[:, ct, :],
                                         hT[:, ct, tc2 * 512:(tc2 + 1) * 512],
                                         start=(ct == 0), stop=(ct == NCT - 1))
                    nc.vector.tensor_copy(
                        out=dst[:, ft % 8, tc2 * 512:(tc2 + 1) * 512], in_=pm[:])

            def emit_S(ic, h):
                po = (h % 2) * D
                cth = h // 2
                pts = []
                for jt in range(4 * ic + 4):
                    vs = max(0, jt * P - ic * 512)
                    pm = ps_s.tile([P, 512], FP32, tag="smm", name=f"s{ic}_{h}_{jt}")
                    nc.tensor.matmul(
                        pm[:, vs:512],
                        qpT[po:po + D, cth, jt * P:(jt + 1) * P],
                        kpT[po:po + D, cth, ic * 512 + vs:(ic + 1) * 512],
                        start=True, stop=True)
                    pt = pt_pool.tile([P, 512], BF16, tag="pt", name=f"pt{ic}_{h}_{jt}")
                    nc.scalar.activation(pt[:, vs:512], pm[:, vs:512], AF.Exp)
                    if jt >= ic * 4:
                        dd = jt * P - ic * 512
                        nc.vector.tensor_tensor(
                            pt[:, dd:dd + P], pt[:, dd:dd + P], tri01[:], op=OP.mult)
                    pts.append(pt)
                return pts

            def emit_PV_mm(ic, h, pts):
                n_jt = 4 * ic + 4
                pvT = ps_pv.tile([D + 1, 512], FP32, tag="pv", name=f"pv{ic}_{h}")
                for jt in range(n_jt):
                    vs = max(0, jt * P - ic * 512)
                    nc.tensor.matmul(pvT[:, vs:512], v_aug[:, jt, h, :],
                                     pts[jt][:, vs:512],
                                     start=(jt == 0), stop=(jt == n_jt - 1))
                rd = sm.tile([1, 512], BF16, tag="rd")
                with nc.allow_low_precision(
                        reason="softmax denom reciprocal feeds bf16 rank-1 bcast"):
                    nc.vector.reciprocal(rd[:], pvT[D:D + 1, :])
                return pvT, rd

            def emit_PV_norm(ic, h, pvT, rd):
                po = (h % 2) * D
                cth = h // 2
                rb = ps_s.tile([P, 512], FP32, tag="smm", name=f"rb{ic}_{h}")
                nc.tensor.matmul(rb[:], ones_row[:], rd[:], start=True, stop=True)
                rb_sb = rb_pool.tile([P, 512], BF16, tag="rbsb")
                nc.vector.tensor_copy(out=rb_sb[:], in_=rb[:])
                nc.vector.tensor_tensor(
                    aoT[po:po + D, cth, ic * 512:(ic + 1) * 512],
                    pvT[0:D, :], rb_sb[po:po + D, :], op=OP.mult)

            pts_q = {}
            pv_q = {}

            def attn_step(ic, it):
                if it < H:
                    pts_q[it] = emit_S(ic, it)
                if 0 <= it - 1 < H:
                    pv_q[it - 1] = emit_PV_mm(ic, it - 1, pts_q.pop(it - 1))
                if 0 <= it - 2 < H:
                    emit_PV_norm(ic, it - 2, *pv_q.pop(it - 2))

            def emit_proj(tt):
                for cc2 in range(2):
                    pm = ps_fc.tile([P, 512], FP32, tag="fcp", name=f"prj{tt}_{cc2}")
                    for ct in range(NCT):
                        nc.tensor.matmul(pm[:], aoT[:, ct, tt * P:(tt + 1) * P],
                                         wp[:, ct, cc2 * 512:(cc2 + 1) * 512],
                                         start=(ct == 0), stop=(ct == NCT - 1))
                    xr = spool.tile([P, 512], FP32, tag="xres")
                    nc.sync.dma_start(
                        xr[:], x_d[tt * P:(tt + 1) * P, cc2 * 512:(cc2 + 1) * 512])
                    nc.vector.tensor_tensor(
                        x2_sb[:, tt, cc2 * 512:(cc2 + 1) * 512], pm[:], xr[:],
                        op=OP.add)

            # ---- phase B: qk projections + attention over BOTH i-chunks in
            # one head pipeline, one qk ft per step spread across all 16
            # steps, and PV(head s-2) / S(head s-1) interleaved per j-tile so
            # exp-gated S psum-slot waits always have independent PE work
            # in front of them. ACT (exp) is the pacer; PE stays dense.
            ftlist = [v for hp in range(8) for v in (hp, 8 + hp)]
            SEQ = [(0, j) for j in range(4)] + [(1, j) for j in range(8)]

            def emit_S_one(ic, h, jt, pts):
                po = (h % 2) * D
                cth = h // 2
                vs = max(0, jt * P - ic * 512)
                pm = ps_s.tile([P, 512], FP32, tag="smm", name=f"s{ic}_{h}_{jt}")
                nc.tensor.matmul(
                    pm[:, vs:512],
                    qpT[po:po + D, cth, jt * P:(jt + 1) * P],
                    kpT[po:po + D, cth, ic * 512 + vs:(ic + 1) * 512],
                    start=True, stop=True)
                pt = pt_pool.tile([P, 512], BF16, tag="pt", name=f"pt{ic}_{h}_{jt}")
                nc.scalar.activation(pt[:, vs:512], pm[:, vs:512], AF.Exp)
                if jt >= ic * 4:
                    dd = jt * P - ic * 512
                    nc.vector.tensor_tensor(
                        pt[:, dd:dd + P], pt[:, dd:dd + P], tri01[:], op=OP.mult)
                pts.append(pt)

            for s in range(H + 4):
                if 1 <= s <= 16:
                    emit_qk_ft(ftlist[s - 1])
                h3 = s - 4
                if 0 <= h3 < H:
                    b0, b1 = pv_q.pop(h3)
                    emit_PV_norm(0, h3, *b0)
                    emit_PV_norm(1, h3, *b1)
                h_pv = s - 3
                h_s = s - 2
                pv_pair = None
                if 0 <= h_pv < H:
                    apair = pts_q.pop(h_pv)
                    pv_pair = [
                        ps_pv.tile([D + 1, 512], FP32, tag="pv", name=f"pv0_{h_pv}"),
                        ps_pv.tile([D + 1, 512], FP32, tag="pv", name=f"pv1_{h_pv}"),
                    ]
                new_pts = ([], []) if 0 <= h_s < H else None
                # PV chains must stay contiguous: matmuls interleaved inside an
                # open PSUM accumulation group hard-fault the exec unit.
                if pv_pair is not None:
                    for ic, jt in SEQ:
                        vs = max(0, jt * P - ic * 512)
                        n_jt = 4 * ic + 4
                        nc.tensor.matmul(pv_pair[ic][:, vs:512],
                                         v_aug[:, jt, h_pv, :],
                                         apair[ic][jt][:, vs:512],
                                         start=(jt == 0), stop=(jt == n_jt - 1))
                if new_pts is not None:
                    for ic, jt in SEQ:
                        emit_S_one(ic, h_s, jt, new_pts[ic])
                if pv_pair is not None:
                    rds = []
                    for ic in range(2):
                        rd = sm.tile([1, 512], BF16, tag="rd")
                        with nc.allow_low_precision(
                                reason="softmax denom recip feeds bf16 bcast"):
                            nc.vector.reciprocal(rd[:], pv_pair[ic][D:D + 1, :])
                        rds.append(rd)
                    pv_q[h_pv] = ((pv_pair[0], rds[0]), (pv_pair[1], rds[1]))
                if new_pts is not None:
                    pts_q[h_s] = new_pts

        # ---- phase D: proj tts 4..7 + LN2 + fc + mlp ----
        if True:
            ps_tr2 = cs.enter_context(
                tc.tile_pool(name="ps_tr2", bufs=2, space="PSUM"))
            ps_fc = cs.enter_context(
                tc.tile_pool(name="ps_fc", bufs=4, space="PSUM"))
            h2_pool = top.enter_context(tc.tile_pool(name="h2Tp", bufs=1))
            mT_pool = top.enter_context(tc.tile_pool(name="mT", bufs=1))
            wf_pool = cs.enter_context(tc.tile_pool(name="wfc", bufs=4))
            h2T = h2_pool.tile([P, NCT, T], BF16, tag="h2T")
            mT = mT_pool.tile([P, NFT, T], BF16, tag="mT")

            def emit_ln2(tt):
                h2 = spool.tile([P, C], BF16, tag="h")
                emit_ln(x2_sb[:, tt, :], h2[:])
                for ct in range(NCT):
                    ptr = ps_tr2.tile([P, P], BF16, tag="tr2")
                    nc.tensor.transpose(ptr[:], h2[:, ct * P:(ct + 1) * P], ident[:])
                    nc.vector.tensor_copy(out=h2T[:, ct, tt * P:(tt + 1) * P],
                                          in_=ptr[:])

            for tt in range(NT):
                emit_proj(tt)
                emit_ln2(tt)

            def emit_fc(ft, tc2):
                wf = wf_pool.tile([P, NCT, P], BF16, tag="wfc", name=f"wfc{tc2}_{ft}")
                nc.sync.dma_start(wf[:], wfc_d[ft])
                pm = ps_fc.tile([P, 512], FP32, tag="fcp", name=f"fc{tc2}_{ft}")
                for ct in range(NCT):
                    nc.tensor.matmul(pm[:], wf[:, ct, :],
                                     h2T[:, ct, tc2 * 512:(tc2 + 1) * 512],
                                     start=(ct == 0), stop=(ct == NCT - 1))
                nc.scalar.activation(mT[:, ft, tc2 * 512:(tc2 + 1) * 512],
                                     pm[:], AF.Gelu)

            for tc2 in range(2):
                for ft in range(NFT):
                    emit_fc(ft, tc2)

            cs.close()  # release mm/aux/wfc psum+sbuf before the 8-bank proj pool
            with ExitStack() as pr_scope:
                wm_pool = pr_scope.enter_context(tc.tile_pool(name="wmp", bufs=3))
                ps_pr = pr_scope.enter_context(
                    tc.tile_pool(name="ps_proj", bufs=8, space="PSUM"))
                for cc2 in range(2):
                    pms = [ps_pr.tile([P, 512], FP32, tag="mproj", name=f"mp{cc2}_{i}")
                           for i in range(NT)]
                    for fg in range(NFT // 4):
                        wm = wm_pool.tile([P, 4, 512], BF16, tag="wmp")
                        nc.sync.dma_start(wm[:], wmp_d[cc2][:, fg * 4:(fg + 1) * 4, :])
                        for fi in range(4):
                            ft = fg * 4 + fi
                            for tt in range(NT):
                                nc.tensor.matmul(pms[tt][:],
                                                 mT[:, ft, tt * P:(tt + 1) * P],
                                                 wm[:, fi, :],
                                                 start=(ft == 0), stop=(ft == NFT - 1))
                    for tt in range(NT):
                        ot = spool.tile([P, 512], FP32, tag="osb")
                        nc.vector.tensor_tensor(
                            ot[:], pms[tt][:], x2_sb[:, tt, cc2 * 512:(cc2 + 1) * 512],
                            op=OP.add)
                        nc.sync.dma_start(
                            out_d[tt * P:(tt + 1) * P, cc2 * 512:(cc2 + 1) * 512], ot[:])


@functools.lru_cache(maxsize=1)
def _compiled():
    nc = bacc.Bacc("TRN2", target_bir_lowering=False, debug=False)
    with tile.TileContext(nc) as tc:
        emit_block(nc, tc)
    nc.compile()
    return nc


def _prepro(inputs):
    f32 = np.float32
    inp = {k: np.asarray(v, f32) for k, v in inputs.items()}
    g1, b1 = inp["ln1_g"], inp["ln1_b"]
    W = inp["attn_w"] * g1[:, None]
    bias_kqv = inp["attn_b"] + b1 @ inp["attn_w"]
    W = W.copy()
    W[:, :C] *= 1.0 / np.sqrt(D)
    bias_kqv = bias_kqv.copy()
    bias_kqv[:C] *= 1.0 / np.sqrt(D)
    assert not np.any(bias_kqv), "nonzero attn bias not supported by this build"
    assert not np.any(inp["attn_proj_b"]) and not np.any(inp["fc_b"]) \
        and not np.any(inp["mlp_proj_b"]), "nonzero biases not supported"

    wqk = np.ascontiguousarray(
        W[:, :2 * C].astype(bf16).reshape(NCT, P, 16, P).transpose(2, 1, 0, 3))
    wv = np.ascontiguousarray(
        W[:, 2 * C:].astype(bf16).reshape(NCT, P, C).transpose(1, 0, 2))
    wproj = np.ascontiguousarray(
        inp["attn_proj_w"].astype(bf16).reshape(NCT, P, C).transpose(1, 0, 2))
    wfc = np.ascontiguousarray(
        (inp["fc_w"] * inp["ln2_g"][:, None]).astype(bf16)
        .reshape(NCT, P, NFT, P).transpose(2, 1, 0, 3))
    assert not np.any(inp["ln2_b"]), "nonzero ln2 bias not supported"
    wmp = np.ascontiguousarray(
        inp["mlp_proj_w"].astype(bf16).reshape(NFT, P, 2, 512).transpose(2, 1, 0, 3))
    ident = np.eye(P, dtype=bf16)
    tri01 = np.triu(np.ones((P, P), np.float32)).astype(bf16)  # 1 where col >= row
    return inp["x"], dict(wqk=wqk, wv=wv, wproj=wproj, wfc=wfc, wmp=wmp,
                          ident=ident, tri01=tri01)


def kernel(**inputs) -> np.ndarray:
    x, weights = _prepro(inputs)
    nc = _compiled()
    in_maps = [{"x": np.ascontiguousarray(x[b]), **weights} for b in range(B)]
    res = run_bass_kernel_spmd(nc, in_maps, list(range(B)))
    return np.stack([res.results[b]["out"] for b in range(B)]).astype(np.float32)



# revision 2
# speedup vs baseline: 1.0475x; 1.0475x over previous
"""Trainium2 Bass kernel for a pre-LN transformer block (B=8,T=1024,C=1024,H=16,FF=4096).

Sharding: pure data-parallel over batch — B=8 equals the 8 NeuronCores, each core
runs the full block on one (T, C) slice; weights are replicated. No collectives.

Per-core layout strategy:
  - LayerNorm in token-major [tok(P), C], gains/biases folded into downstream
    weights on the host; normalized activations cast to fp8e4 and PE-transposed
    to feature-major h^T [C(P), tok] for use as matmul operands.
  - QKV in fp8 (DoubleRow pairs, 2x PE throughput): k-proj/q-proj produced
    feature-major (out^T = W_tile.T @ h^T) as bf16, v produced token-major fp8
    with a per-head ones column appended (softmax denominators). The reference
    computes wei[i,j] = k_i . q_j; 1/sqrt(D) is folded into the exp scale.
  - Attention: S^T[j,i] tiles via K=64 bf16 matmuls, causal tile skipping,
    exp(0.125*s) without max-subtraction (scores are O(3)), multiplicative
    triangular mask on diagonal tiles, probabilities stored fp8 in per-(ic,h)
    [P, n_jt, 512] tiles so PV runs as fp8 DoubleRow j-tile pairs:
    out[i,65] = P^T_pair.T @ v_aug, column 64 = softmax denominator;
    normalize via PE broadcast of the denom row + reciprocal_approx_fast.
  - attn-proj in fp8 DoubleRow; fc / mlp-proj as bf16 tiled matmuls with fp32
    PSUM accum (fp8 there would blow the 2e-2 error budget); residuals fp32.
"""

import functools

import ml_dtypes
import numpy as np

import concourse.bass as bass
import concourse.mybir as mybir
import concourse.tile as tile
from concourse import bacc
from concourse.bass_utils import run_bass_kernel_spmd

bf16 = ml_dtypes.bfloat16
f8e4 = ml_dtypes.float8_e4m3
FP32 = mybir.dt.float32
BF16 = mybir.dt.bfloat16
F8 = mybir.dt.float8e4
AX = mybir.AxisListType
OP = mybir.AluOpType
AF = mybir.ActivationFunctionType
DR = mybir.MatmulPerfMode.DoubleRow

B, T, C, H = 8, 1024, 1024, 16
D = C // H          # 64
FF = 4 * C          # 4096
P = 128
NT = T // P         # 8 token tiles
NCT = C // P        # 8 channel tiles
NFT = FF // P       # 32 ff tiles
SCALE = 0.125       # 1/sqrt(D), folded into exp


def emit_block(nc, tc):
    """Emit the whole per-core transformer block program.

    Emission order is a global software pipeline: the qk projections are
    interleaved with the attention head pipeline so the ACT-bound softmax exp
    always has dense PE work running beside it.
    """
    x_d = nc.dram_tensor("x", [T, C], FP32, kind="ExternalInput").ap()
    wqk_d = nc.dram_tensor("wqk", [16, P, NCT, P], F8, kind="ExternalInput").ap()
    wv_d = nc.dram_tensor("wv", [P, NCT, C], F8, kind="ExternalInput").ap()
    wproj_d = nc.dram_tensor("wproj", [P, NCT, C], F8, kind="ExternalInput").ap()
    wfc_d = nc.dram_tensor("wfc", [NFT, P, NCT, P], BF16, kind="ExternalInput").ap()
    wmp_d = nc.dram_tensor("wmp", [2, P, NFT, 512], BF16, kind="ExternalInput").ap()
    ident_d = nc.dram_tensor("ident", [P, P], BF16, kind="ExternalInput").ap()
    tri_d = nc.dram_tensor("tri01", [P, P], F8, kind="ExternalInput").ap()
    out_d = nc.dram_tensor("out", [T, C], FP32, kind="ExternalOutput").ap()

    from contextlib import ExitStack
    with ExitStack() as top:
        cpool = top.enter_context(tc.tile_pool(name="const", bufs=1))
        ppool = top.enter_context(tc.tile_pool(name="persist", bufs=1))
        spool = top.enter_context(tc.tile_pool(name="stream", bufs=2))
        sm = top.enter_context(tc.tile_pool(name="small", bufs=4))
        aoT_pool = top.enter_context(tc.tile_pool(name="aoT", bufs=1))
        wp_pool = top.enter_context(tc.tile_pool(name="wproj", bufs=1))
        cs = top.enter_context(ExitStack())
        ps_mm = cs.enter_context(tc.tile_pool(name="ps_mm", bufs=2, space="PSUM"))

        ident = cpool.tile([P, P], BF16, tag="ident")
        tri01 = cpool.tile([P, P], F8, tag="tri01")
        zero1 = cpool.tile([P, 1], FP32, tag="zero1")
        eps1 = cpool.tile([P, 1], FP32, tag="eps1")
        ones_row = cpool.tile([1, P], BF16, tag="ones_row")

        x2_sb = ppool.tile([P, NT, C], FP32, tag="x2")
        aoT = aoT_pool.tile([P, NCT, T], F8, tag="aoT")
        wp = wp_pool.tile([P, NCT, C], F8, tag="wproj")

        def emit_ln(x_tile, h_out):
            width = x_tile.shape[-1]
            s = sm.tile([P, 1], FP32, tag="ln_s")
            ssq = sm.tile([P, 1], FP32, tag="ln_ssq")
            mu = sm.tile([P, 1], FP32, tag="ln_mu")
            var = sm.tile([P, 1], FP32, tag="ln_var")
            std = sm.tile([P, 1], FP32, tag="ln_std")
            rstd = sm.tile([P, 1], FP32, tag="ln_rstd")
            sq = spool.tile([P, C], FP32, tag="ln_sq")
            nc.vector.reduce_sum(s, x_tile, axis=AX.X)
            nc.scalar.activation(sq[:, :width], x_tile, AF.Square, accum_out=ssq)
            nc.vector.tensor_scalar_mul(mu, s, 1.0 / width)
            nc.vector.tensor_scalar_mul(var, ssq, 1.0 / width)
            nc.vector.tensor_tensor(s, mu, mu, op=OP.mult)
            nc.vector.tensor_tensor(var, var, s, op=OP.subtract)
            nc.scalar.activation(std, var, AF.Sqrt, bias=1e-5)
            nc.vector.reciprocal(rstd, std)
            nc.vector.tensor_scalar(h_out, x_tile, scalar1=mu, scalar2=rstd,
                                    op0=OP.subtract, op1=OP.mult)

        with ExitStack() as attn_scope:
            ps_pv = attn_scope.enter_context(
                tc.tile_pool(name="ps_pv", bufs=2, space="PSUM"))
            ps_s = attn_scope.enter_context(
                tc.tile_pool(name="ps_s", bufs=4, space="PSUM"))
            qk_pool = attn_scope.enter_context(tc.tile_pool(name="qk", bufs=1))
            v_pool = attn_scope.enter_context(tc.tile_pool(name="v", bufs=1))
            hT_pool = attn_scope.enter_context(tc.tile_pool(name="hTp", bufs=1))
            wq_pool = attn_scope.enter_context(tc.tile_pool(name="wqkv", bufs=2))
            wv_pool = attn_scope.enter_context(tc.tile_pool(name="wvp", bufs=1))
            pt0_pool = attn_scope.enter_context(tc.tile_pool(name="pt0", bufs=3))
            pt1_pool = attn_scope.enter_context(tc.tile_pool(name="pt1", bufs=3))
            rb_pool = attn_scope.enter_context(tc.tile_pool(name="rbp", bufs=2))

            qpT = qk_pool.tile([P, NCT, T], BF16, tag="qpT")  # q-proj^T (key role)
            kpT = qk_pool.tile([P, NCT, T], BF16, tag="kpT")  # k-proj^T (query role)
            v_aug = v_pool.tile([P, NT, H, D + 1], F8, tag="vaug")
            hT = hT_pool.tile([P, NCT, T], F8, tag="hT")

            # x tiles first (LN1 critical path), then constants + weights.
            xts = []
            for tt in range(NT):
                xt = spool.tile([P, C], FP32, tag="xin", name=f"xin{tt}")
                nc.sync.dma_start(xt[:], x_d[tt * P:(tt + 1) * P, :])
                xts.append(xt)
            nc.sync.dma_start(ident[:], ident_d)
            nc.sync.dma_start(tri01[:], tri_d)
            nc.gpsimd.memset(zero1[:], 0.0)
            nc.gpsimd.memset(eps1[:], 1e-5)
            nc.gpsimd.memset(ones_row[:], 1.0)
            nc.const_aps.aps[(FP32, 0.0)] = zero1[:]
            nc.const_aps.aps[(FP32, 1e-5)] = eps1[:]
            nc.gpsimd.memset(v_aug[:, :, :, D:D + 1], 1.0)
            wv_sb = wv_pool.tile([P, NCT, C], F8, tag="wv")
            nc.sync.dma_start(wv_sb[:], wv_d)
            nc.sync.dma_start(wp[:], wproj_d)

            # ---- phase A: LN1 + transpose + v projection per token tile ----
            for tt in range(NT):
                ht = spool.tile([P, C], BF16, tag="h")
                emit_ln(xts[tt][:], ht[:])
                for ct in range(NCT):
                    ptr = ps_s.tile([P, P], BF16, tag="smm")
                    nc.tensor.transpose(ptr[:], ht[:, ct * P:(ct + 1) * P], ident[:])
                    nc.vector.tensor_copy(out=hT[:, ct, tt * P:(tt + 1) * P], in_=ptr[:])
                for fc2 in range(2):
                    pm = ps_mm.tile([P, 512], FP32, tag="mm")
                    for cp in range(NCT // 2):
                        nc.tensor.matmul(
                            pm[:], hT[:, 2 * cp:2 * cp + 2, tt * P:(tt + 1) * P],
                            wv_sb[:, 2 * cp:2 * cp + 2, fc2 * 512:(fc2 + 1) * 512],
                            start=(cp == 0), stop=(cp == NCT // 2 - 1),
                            perf_mode=DR)
                    nc.vector.tensor_copy(
                        out=v_aug[:, tt, fc2 * 8:(fc2 + 1) * 8, 0:D],
                        in_=pm[:].rearrange("p (h d) -> p h d", d=D))

            def emit_qk_ft(ft):
                wt = wq_pool.tile([P, NCT, P], F8, tag="wqk", name=f"wqk{ft}")
                nc.sync.dma_start(wt[:], wqk_d[ft])
                dst = kpT if ft < 8 else qpT
                for tc2 in range(2):
                    pm = ps_mm.tile([P, 512], FP32, tag="mm", name=f"qk{ft}_{tc2}")
                    for cp in range(NCT // 2):
                        nc.tensor.matmul(
                            pm[:], wt[:, 2 * cp:2 * cp + 2, :],
                            hT[:, 2 * cp:2 * cp + 2, tc2 * 512:(tc2 + 1) * 512],
                            start=(cp == 0), stop=(cp == NCT // 2 - 1),
                            perf_mode=DR)
                    nc.vector.tensor_copy(
                        out=dst[:, ft % 8, tc2 * 512:(tc2 + 1) * 512], in_=pm[:])

            def emit_PV_norm(ic, h, pvT):
                po = (h % 2) * D
                cth = h // 2
                den = sm.tile([1, 512], BF16, tag="den")
                nc.scalar.activation(den[:], pvT[D:D + 1, :], AF.Copy)
                rb = ps_s.tile([P, 512], FP32, tag="smm", name=f"rb{ic}_{h}")
                nc.tensor.matmul(rb[:], ones_row[:], den[:], start=True, stop=True)
                rcp = rb_pool.tile([P, 512], FP32, tag="rcp")
                nc.vector.reciprocal_approx_fast(rcp[:], rb[:])
                nc.vector.tensor_tensor(
                    aoT[po:po + D, cth, ic * 512:(ic + 1) * 512],
                    pvT[0:D, :], rcp[po:po + D, :], op=OP.mult)

            def emit_proj(tt):
                for cc2 in range(2):
                    pm = ps_fc.tile([P, 512], FP32, tag="fcp", name=f"prj{tt}_{cc2}")
                    for cp in range(NCT // 2):
                        nc.tensor.matmul(
                            pm[:], aoT[:, 2 * cp:2 * cp + 2, tt * P:(tt + 1) * P],
                            wp[:, 2 * cp:2 * cp + 2, cc2 * 512:(cc2 + 1) * 512],
                            start=(cp == 0), stop=(cp == NCT // 2 - 1),
                            perf_mode=DR)
                    xr = spool.tile([P, 512], FP32, tag="xres")
                    nc.sync.dma_start(
                        xr[:], x_d[tt * P:(tt + 1) * P, cc2 * 512:(cc2 + 1) * 512])
                    nc.vector.tensor_tensor(
                        x2_sb[:, tt, cc2 * 512:(cc2 + 1) * 512], pm[:], xr[:],
                        op=OP.add)

            # ---- phase B: qk projections + attention over BOTH i-chunks in
            # one head pipeline, one qk ft per step spread across all 16
            # steps, and PV(head s-2) / S(head s-1) interleaved per j-tile so
            # exp-gated S psum-slot waits always have independent PE work
            # in front of them. ACT (exp) is the pacer; PE stays dense.
            ftlist = [v for hp in range(8) for v in (hp, 8 + hp)]
            SEQ = [(0, j) for j in range(4)] + [(1, j) for j in range(8)]

            def emit_S_one(ic, h, jt, ptile):
                po = (h % 2) * D
                cth = h // 2
                vs = max(0, jt * P - ic * 512)
                pm = ps_s.tile([P, 512], FP32, tag="smm", name=f"s{ic}_{h}_{jt}")
                nc.tensor.matmul(
                    pm[:, vs:512],
                    qpT[po:po + D, cth, jt * P:(jt + 1) * P],
                    kpT[po:po + D, cth, ic * 512 + vs:(ic + 1) * 512],
                    start=True, stop=True)
                nc.scalar.activation(ptile[:, jt, vs:512], pm[:, vs:512], AF.Exp,
                                     scale=SCALE)
                if jt >= ic * 4:
                    dd = jt * P - ic * 512
                    nc.vector.tensor_tensor(
                        ptile[:, jt, dd:dd + P], ptile[:, jt, dd:dd + P], tri01[:],
                        op=OP.mult)
                if jt % 2 == 1 and vs > 0:
                    # zero the beyond-causal gap so the DoubleRow PV pair can
                    # read the union width [vs_even:512] of both tiles
                    nc.gpsimd.memset(ptile[:, jt, vs - P:vs], 0.0)

            pts_q = {}
            pv_q = {}
            for s in range(H + 4):
                if 1 <= s <= 16:
                    emit_qk_ft(ftlist[s - 1])
                h3 = s - 4
                if 0 <= h3 < H:
                    b0, b1 = pv_q.pop(h3)
                    emit_PV_norm(0, h3, b0)
                    emit_PV_norm(1, h3, b1)
                h_pv = s - 3
                h_s = s - 2
                pv_pair = None
                if 0 <= h_pv < H:
                    apair = pts_q.pop(h_pv)
                    pv_pair = [
                        ps_pv.tile([D + 1, 512], FP32, tag="pv", name=f"pv0_{h_pv}"),
                        ps_pv.tile([D + 1, 512], FP32, tag="pv", name=f"pv1_{h_pv}"),
                    ]
                new_pts = None
                if 0 <= h_s < H:
                    new_pts = (
                        pt0_pool.tile([P, 4, 512], F8, tag="pt0", name=f"pt0_{h_s}"),
                        pt1_pool.tile([P, 8, 512], F8, tag="pt1", name=f"pt1_{h_s}"),
                    )
                # PV chains must stay contiguous: matmuls interleaved inside an
                # open PSUM accumulation group hard-fault the exec unit.
                if pv_pair is not None:
                    for ic in range(2):
                        n_pair = 2 * ic + 2
                        for pj in range(n_pair):
                            vs = max(0, 2 * pj * P - ic * 512)
                            nc.tensor.matmul(
                                pv_pair[ic][:, vs:512],
                                v_aug[:, 2 * pj:2 * pj + 2, h_pv, :],
                                apair[ic][:, 2 * pj:2 * pj + 2, vs:512],
                                start=(pj == 0), stop=(pj == n_pair - 1),
                                perf_mode=DR)
                if new_pts is not None:
                    for ic, jt in SEQ:
                        emit_S_one(ic, h_s, jt, new_pts[ic])
                if pv_pair is not None:
                    pv_q[h_pv] = (pv_pair[0], pv_pair[1])
                if new_pts is not None:
                    pts_q[h_s] = new_pts

        # ---- phase D: proj tts + LN2 + fc + mlp ----
        if True:
            ps_tr2 = cs.enter_context(
                tc.tile_pool(name="ps_tr2", bufs=2, space="PSUM"))
            ps_fc = cs.enter_context(
                tc.tile_pool(name="ps_fc", bufs=4, space="PSUM"))
            h2_pool = top.enter_context(tc.tile_pool(name="h2Tp", bufs=1))
            mT_pool = top.enter_context(tc.tile_pool(name="mT", bufs=1))
            wf_pool = cs.enter_context(tc.tile_pool(name="wfc", bufs=4))
            h2T = h2_pool.tile([P, NCT, T], BF16, tag="h2T")
            mT = mT_pool.tile([P, NFT, T], BF16, tag="mT")

            def emit_ln2(tt):
                h2 = spool.tile([P, C], BF16, tag="h")
                emit_ln(x2_sb[:, tt, :], h2[:])
                for ct in range(NCT):
                    ptr = ps_tr2.tile([P, P], BF16, tag="tr2")
                    nc.tensor.transpose(ptr[:], h2[:, ct * P:(ct + 1) * P], ident[:])
                    nc.vector.tensor_copy(out=h2T[:, ct, tt * P:(tt + 1) * P],
                                          in_=ptr[:])

            for tt in range(NT):
                emit_proj(tt)
                emit_ln2(tt)

            def emit_fc(ft, tc2):
                wf = wf_pool.tile([P, NCT, P], BF16, tag="wfc", name=f"wfc{tc2}_{ft}")
                nc.sync.dma_start(wf[:], wfc_d[ft])
                pm = ps_fc.tile([P, 512], FP32, tag="fcp", name=f"fc{tc2}_{ft}")
                for ct in range(NCT):
                    nc.tensor.matmul(pm[:], wf[:, ct, :],
                                     h2T[:, ct, tc2 * 512:(tc2 + 1) * 512],
                                     start=(ct == 0), stop=(ct == NCT - 1))
                nc.scalar.activation(mT[:, ft, tc2 * 512:(tc2 + 1) * 512],
                                     pm[:], AF.Gelu)

            for tc2 in range(2):
                for ft in range(NFT):
                    emit_fc(ft, tc2)

            cs.close()  # release mm/aux/wfc psum+sbuf before the 8-bank proj pool
            with ExitStack() as pr_scope:
                wm_pool = pr_scope.enter_context(tc.tile_pool(name="wmp", bufs=3))
                ps_pr = pr_scope.enter_context(
                    tc.tile_pool(name="ps_proj", bufs=8, space="PSUM"))
                for cc2 in range(2):
                    pms = [ps_pr.tile([P, 512], FP32, tag="mproj", name=f"mp{cc2}_{i}")
                           for i in range(NT)]
                    for fg in range(NFT // 4):
                        wm = wm_pool.tile([P, 4, 512], BF16, tag="wmp")
                        nc.sync.dma_start(wm[:], wmp_d[cc2][:, fg * 4:(fg + 1) * 4, :])
                        for fi in range(4):
                            ft = fg * 4 + fi
                            for tt in range(NT):
                                nc.tensor.matmul(pms[tt][:],
                                                 mT[:, ft, tt * P:(tt + 1) * P],
                                                 wm[:, fi, :],
                                                 start=(ft == 0), stop=(ft == NFT - 1))
                    for tt in range(NT):
                        ot = spool.tile([P, 512], FP32, tag="osb")
                        nc.vector.tensor_tensor(
                            ot[:], pms[tt][:], x2_sb[:, tt, cc2 * 512:(cc2 + 1) * 512],
                            op=OP.add)
                        nc.sync.dma_start(
                            out_d[tt * P:(tt + 1) * P, cc2 * 512:(cc2 + 1) * 512], ot[:])


@functools.lru_cache(maxsize=1)
def _compiled():
    nc = bacc.Bacc("TRN2", target_bir_lowering=False, debug=False)
    with tile.TileContext(nc) as tc:
        emit_block(nc, tc)
    nc.compile()
    return nc


def _prepro(inputs):
    f32 = np.float32
    inp = {k: np.asarray(v, f32) for k, v in inputs.items()}
    g1, b1 = inp["ln1_g"], inp["ln1_b"]
    W = inp["attn_w"] * g1[:, None]
    bias_kqv = inp["attn_b"] + b1 @ inp["attn_w"]
    assert not np.any(bias_kqv), "nonzero attn bias not supported by this build"
    assert not np.any(inp["attn_proj_b"]) and not np.any(inp["fc_b"]) \
        and not np.any(inp["mlp_proj_b"]), "nonzero biases not supported"

    wqk = np.ascontiguousarray(
        W[:, :2 * C].astype(f8e4).reshape(NCT, P, 16, P).transpose(2, 1, 0, 3))
    wv = np.ascontiguousarray(
        W[:, 2 * C:].astype(f8e4).reshape(NCT, P, C).transpose(1, 0, 2))
    wproj = np.ascontiguousarray(
        inp["attn_proj_w"].astype(f8e4).reshape(NCT, P, C).transpose(1, 0, 2))
    wfc = np.ascontiguousarray(
        (inp["fc_w"] * inp["ln2_g"][:, None]).astype(bf16)
        .reshape(NCT, P, NFT, P).transpose(2, 1, 0, 3))
    assert not np.any(inp["ln2_b"]), "nonzero ln2 bias not supported"
    wmp = np.ascontiguousarray(
        inp["mlp_proj_w"].astype(bf16).reshape(NFT, P, 2, 512).transpose(2, 1, 0, 3))
    ident = np.eye(P, dtype=bf16)
    tri01 = np.triu(np.ones((P, P), np.float32)).astype(f8e4)  # 1 where col >= row
    return inp["x"], dict(wqk=wqk, wv=wv, wproj=wproj, wfc=wfc, wmp=wmp,
                          ident=ident, tri01=tri01)


def kernel(**inputs) -> np.ndarray:
    x, weights = _prepro(inputs)
    nc = _compiled()
    in_maps = [{"x": np.ascontiguousarray(x[b]), **weights} for b in range(B)]
    res = run_bass_kernel_spmd(nc, in_maps, list(range(B)))
    return np.stack([res.results[b]["out"] for b in range(B)]).astype(np.float32)
